# revision 10
# baseline (speedup 1.0000x reference)
"""Trainium2 Bass kernel for nn_DirectedEdgeDecoder (gnn_message_passing).

  out[e] = W2 . relu(concat(z1[row_e], z2[col_e]) @ W1 + b1) + b2

Key algebraic rewrite: the first MLP layer is linear over the concat, so
  concat(z1[r], z2[c]) @ W1 = z1[r] @ W1[:D] + z2[c] @ W1[D:]
which lets us precompute per-node projections u1 = z1 @ W1[:D] + b1 and
u2 = z2 @ W1[D:] (8 floats per node instead of 128), reducing the random
per-edge gather traffic by 16x.

Two SPMD launches over 8 NeuronCores:
  Kernel A (node-sharded): each core computes u1/u2 for 12500 nodes
    (reads only its 1/8 slice of z1/z2).
  Host: concatenates per-core u tables into one U table (no math, layout only).
  Kernel B (edge-sharded): each core gathers U rows for its 100000 edges via
    indirect DMA (32B rows) and applies relu + the W2 reduction.
"""
import numpy as np
import concourse.bass as bass
import concourse.mybir as mybir
import concourse.tile as tile
from concourse import bacc
from concourse.bass_utils import run_bass_kernel_spmd

P = 128          # partitions
N_CORES = 8
N_NODES = 100000
N_EDGES = 800000
D = 128
H = 8

NC_NODES = N_NODES // N_CORES          # 12500 nodes per core
KN = 98                                # node chunks per core
NP = KN * P                            # 12544 padded nodes per core
EC = N_EDGES // N_CORES                # 100000 edges per core
WB = 98                                # edge slots per partition per block
NB = 8                                 # edge blocks
EW = NB * WB                           # 784 edge slots per partition
EP = P * EW                            # 100352 padded edges per core
U_ROWS = 2 * N_CORES * NP              # 200704

f32 = mybir.dt.float32
i32 = mybir.dt.int32
i64 = mybir.dt.int64


def _new_nc():
    # bacc.Bacc so compile() runs generate_event_semaphores -- this walrus
    # build allows at most 1 sync wait per instruction.
    return bacc.Bacc(
        "TRN2", target_bir_lowering=False, debug=False, num_devices=N_CORES
    )


# ---------------------------------------------------------------- kernel A

def build_precompute():
    """Per-core: u[t] = zT[t].T @ W1[t*128:(t+1)*128] (+ b1 if t == 0).

    Inputs : z1T [128, NP] f32 (z1 shard, transposed, padded), z2T likewise,
             W1 [256, 8] f32, b1 [1, 8] f32
    Output : u [2, NP, 8] f32 -- row r = p*KN + k holds node m = k*128 + p
    """
    nc = _new_nc()
    z1T = nc.declare_dram_parameter("z1T", [P, NP], f32, isOutput=False)
    z2T = nc.declare_dram_parameter("z2T", [P, NP], f32, isOutput=False)
    W1 = nc.declare_dram_parameter("W1", [2 * D, H], f32, isOutput=False)
    b1 = nc.declare_dram_parameter("b1", [1, H], f32, isOutput=False)
    u = nc.declare_dram_parameter("u", [2, NP, H], f32, isOutput=True)

    CH = 14                  # k-chunks per load
    CW = CH * P              # 1792 columns per load
    NLOAD = KN // CH         # 7 loads per table

    with tile.TileContext(nc) as tc:
        with (
            tc.tile_pool(name="const", bufs=1) as const_pool,
            tc.tile_pool(name="zin", bufs=3) as zin_pool,
            tc.tile_pool(name="acc", bufs=2) as acc_pool,
            tc.tile_pool(name="psum", bufs=4, space="PSUM") as psum_pool,
        ):
            w1sb = const_pool.tile([P, 2 * H], f32)   # [:, t*H:(t+1)*H] = half t
            for t in range(2):
                nc.sync.dma_start(
                    out=w1sb[:, t * H:(t + 1) * H], in_=W1[t * P:(t + 1) * P, :]
                )
            b1sb = const_pool.tile([P, H], f32)
            nc.sync.dma_start(out=b1sb[:], in_=b1[:].to_broadcast([P, H]))

            for t, zT in enumerate((z1T, z2T)):
                u_acc = acc_pool.tile([P, KN * H], f32, tag="u_acc")
                for j in range(NLOAD):
                    ztile = zin_pool.tile([P, CW], f32, tag="ztile")
                    nc.sync.dma_start(out=ztile[:], in_=zT[:, j * CW:(j + 1) * CW])
                    ps = psum_pool.tile([P, CH * H], f32, tag="ps")
                    for i in range(CH):
                        nc.tensor.matmul(
                            out=ps[:, i * H:(i + 1) * H],
                            lhsT=ztile[:, i * P:(i + 1) * P],
                            rhs=w1sb[:, t * H:(t + 1) * H],
                            start=True, stop=True,
                        )
                    if t == 0:
                        # fold b1 into u1 during the PSUM->SBUF move
                        nc.vector.tensor_tensor(
                            out=u_acc[:, j * CH * H:(j + 1) * CH * H],
                            in0=ps[:].rearrange("p (c h) -> p c h", h=H),
                            in1=b1sb[:].unsqueeze(1).to_broadcast([P, CH, H]),
                            op=mybir.AluOpType.add,
                        )
                    else:
                        nc.vector.tensor_copy(
                            out=u_acc[:, j * CH * H:(j + 1) * CH * H], in_=ps[:]
                        )
                nc.sync.dma_start(
                    out=u[t].rearrange("(p k) h -> p (k h)", p=P),
                    in_=u_acc[:],
                )
    nc.compile()
    return nc


# ---------------------------------------------------------------- kernel B

def build_gather():
    """Per-core: out[p, j] = W2 . relu(U[idx1[p,j]] + U[idx2[p,j]]) + b2

    One indirect DMA per idx column (the only indirect-DMA shape this
    walrus/ucode build handles correctly is one index per partition,
    gathering a contiguous row per partition).

    Inputs : U [U_ROWS, 8] f32 (replicated), idx1/idx2 [128, 784] i32,
             W2 [1, 8] f32, b2 [1, 1] f32
    Output : out [128, 784] f32   (edge e = j*128 + p at [p, j])
    """
    nc = _new_nc()
    U = nc.declare_dram_parameter("U", [U_ROWS, H], f32, isOutput=False)
    idx1 = nc.declare_dram_parameter("idx1", [P, EW], i32, isOutput=False)
    idx2 = nc.declare_dram_parameter("idx2", [P, EW], i32, isOutput=False)
    W2 = nc.declare_dram_parameter("W2", [1, H], f32, isOutput=False)
    b2 = nc.declare_dram_parameter("b2", [1, 1], f32, isOutput=False)
    out = nc.declare_dram_parameter("out", [P, EW], f32, isOutput=True)

    with tile.TileContext(nc) as tc:
        with (
            tc.tile_pool(name="const", bufs=1) as const_pool,
            tc.tile_pool(name="big", bufs=1) as big_pool,
        ):
            idx1s = const_pool.tile([P, EW], i32)
            nc.sync.dma_start(out=idx1s[:], in_=idx1[:])
            idx2s = const_pool.tile([P, EW], i32)
            nc.sync.dma_start(out=idx2s[:], in_=idx2[:])
            w2sb = const_pool.tile([P, H], f32)
            nc.sync.dma_start(out=w2sb[:], in_=W2[:].to_broadcast([P, H]))
            b2sb = const_pool.tile([P, 1], f32)
            nc.sync.dma_start(out=b2sb[:], in_=b2[:].to_broadcast([P, 1]))

            X1 = big_pool.tile([P, EW * H], f32, tag="X1")
            X2 = big_pool.tile([P, EW * H], f32, tag="X2")
            for j in range(EW):
                nc.gpsimd.indirect_dma_start(
                    out=X1[:, j * H:(j + 1) * H],
                    out_offset=None,
                    in_=U[:],
                    in_offset=bass.IndirectOffsetOnAxis(
                        ap=idx1s[:, j:j + 1], axis=0
                    ),
                )
                nc.gpsimd.indirect_dma_start(
                    out=X2[:, j * H:(j + 1) * H],
                    out_offset=None,
                    in_=U[:],
                    in_offset=bass.IndirectOffsetOnAxis(
                        ap=idx2s[:, j:j + 1], axis=0
                    ),
                )
            nc.vector.tensor_tensor(
                out=X1[:], in0=X1[:], in1=X2[:], op=mybir.AluOpType.add
            )
            nc.scalar.activation(
                out=X1[:], in_=X1[:], func=mybir.ActivationFunctionType.Relu
            )
            nc.vector.tensor_tensor(
                out=X1[:].rearrange("p (w h) -> p w h", h=H),
                in0=X1[:].rearrange("p (w h) -> p w h", h=H),
                in1=w2sb[:].unsqueeze(1).to_broadcast([P, EW, H]),
                op=mybir.AluOpType.mult,
            )
            out_acc = const_pool.tile([P, EW], f32)
            nc.vector.tensor_reduce(
                out=out_acc[:],
                in_=X1[:].rearrange("p (w h) -> p w h", h=H),
                axis=mybir.AxisListType.X,
                op=mybir.AluOpType.add,
            )
            nc.vector.tensor_tensor(
                out=out_acc[:],
                in0=out_acc[:],
                in1=b2sb[:].to_broadcast([P, EW]),
                op=mybir.AluOpType.add,
            )
            nc.sync.dma_start(out=out[:], in_=out_acc[:])
    nc.compile()
    return nc


# ---------------------------------------------------------------- host glue

def edge_layout(a):
    """[EC] int array -> [128, 784] per-core layout (edge e = j*128+p at [p, j])."""
    a = np.pad(a, (0, EP - EC))
    return np.ascontiguousarray(a.reshape(EW, P).T)


def inv_edge_layout(o):
    """[128, 784] kernel output -> [EC] edge-ordered values."""
    return np.ascontiguousarray(o.T.reshape(EP)[:EC])


def node_to_urow(n, table):
    """Original node ids -> U row ids for table 0 (u1) or 1 (u2)."""
    c = n // NC_NODES
    m = n % NC_NODES
    return table * (N_CORES * NP) + c * NP + (m % P) * KN + m // P


def prep_precompute_inputs(z1, z2, W1, b1):
    W1 = np.ascontiguousarray(W1, dtype=np.float32)
    b1 = np.ascontiguousarray(b1, dtype=np.float32).reshape(1, H)
    in_maps = []
    for c in range(N_CORES):
        m = {}
        for name, z in (("z1T", z1), ("z2T", z2)):
            sh = np.zeros((NP, D), dtype=np.float32)
            sh[:NC_NODES] = z[c * NC_NODES:(c + 1) * NC_NODES]
            m[name] = np.ascontiguousarray(sh.T)
        m["W1"] = W1
        m["b1"] = b1
        in_maps.append(m)
    return in_maps


def prep_gather_inputs(U, edge_index, W2, b2):
    row = node_to_urow(np.asarray(edge_index[0], dtype=np.int64), 0)
    col = node_to_urow(np.asarray(edge_index[1], dtype=np.int64), 1)
    W2 = np.ascontiguousarray(np.asarray(W2, dtype=np.float32).reshape(H)[None, :])
    b2 = np.ascontiguousarray(np.asarray(b2, dtype=np.float32)).reshape(1, 1)
    in_maps = []
    for c in range(N_CORES):
        sl = slice(c * EC, (c + 1) * EC)
        in_maps.append({
            "U": U,
            "idx1": edge_layout(row[sl]).astype(np.int32),
            "idx2": edge_layout(col[sl]).astype(np.int32),
            "W2": W2,
            "b2": b2,
        })
    return in_maps


def assemble_u(results):
    parts = [results[c]["u"][0] for c in range(N_CORES)]
    parts += [results[c]["u"][1] for c in range(N_CORES)]
    return np.ascontiguousarray(np.concatenate(parts, axis=0))


def assemble_out(results):
    outs = [inv_edge_layout(results[c]["out"]) for c in range(N_CORES)]
    return np.concatenate(outs, axis=0)[:, None].astype(np.float32)


# ---------------------------------------------------------------- entry

_CACHE = {}


def _get_kernels():
    if "a" not in _CACHE:
        _CACHE["a"] = build_precompute()
        _CACHE["b"] = build_gather()
    return _CACHE["a"], _CACHE["b"]


def run_two_phase(z1, z2, edge_index, W1, b1, W2, b2, trace=False):
    """Returns (output [N_EDGES, 1] f32, results_a, results_b)."""
    nc_a, nc_b = _get_kernels()
    core_ids = list(range(N_CORES))
    in_maps_a = prep_precompute_inputs(z1, z2, W1, b1)
    res_a = run_bass_kernel_spmd(nc_a, in_maps_a, core_ids, trace=trace)
    U = assemble_u(res_a.results)
    in_maps_b = prep_gather_inputs(U, edge_index, W2, b2)
    res_b = run_bass_kernel_spmd(nc_b, in_maps_b, core_ids, trace=trace)
    return assemble_out(res_b.results), res_a, res_b


def kernel(z1, z2, edge_index, W1, b1, W2, b2):
    z1 = np.asarray(z1, dtype=np.float32)
    z2 = np.asarray(z2, dtype=np.float32)
    edge_index = np.asarray(edge_index)
    out, _, _ = run_two_phase(z1, z2, edge_index, W1, b1, W2, b2)
    return out


# revision 11
# speedup vs baseline: 1.0258x; 1.0258x over previous
"""Trainium2 Bass kernel for nn_DirectedEdgeDecoder (gnn_message_passing).

  out[e] = W2 . relu(concat(z1[row_e], z2[col_e]) @ W1 + b1) + b2

Key algebraic rewrite: the first MLP layer is linear over the concat, so
  concat(z1[r], z2[c]) @ W1 = z1[r] @ W1[:D] + z2[c] @ W1[D:]
which lets us precompute per-node projections u1 = z1 @ W1[:D] + b1 and
u2 = z2 @ W1[D:] (8 floats per node instead of 128), reducing the random
per-edge gather traffic by 16x.

Two SPMD launches over 8 NeuronCores:
  Kernel A (node-sharded): each core computes u1/u2 for 12500 nodes
    (reads only its 1/8 slice of z1/z2).
  Host: concatenates per-core u tables into one U table (no math, layout only).
  Kernel B (edge-sharded): each core gathers U rows for its 100000 edges via
    indirect DMA (32B rows) and applies relu + the W2 reduction.
"""
import numpy as np
import concourse.bass as bass
import concourse.mybir as mybir
import concourse.tile as tile
from concourse import bacc
from concourse.bass_utils import run_bass_kernel_spmd

P = 128          # partitions
N_CORES = 8
N_NODES = 100000
N_EDGES = 800000
D = 128
H = 8

NC_NODES = N_NODES // N_CORES          # 12500 nodes per core
KN = 98                                # node chunks per core
NP = KN * P                            # 12544 padded nodes per core
EC = N_EDGES // N_CORES                # 100000 edges per core
WB = 98                                # edge slots per partition per block
NB = 8                                 # edge blocks
EW = NB * WB                           # 784 edge slots per partition
EP = P * EW                            # 100352 padded edges per core
U_ROWS = 2 * N_CORES * NP              # 200704

f32 = mybir.dt.float32
i32 = mybir.dt.int32
i64 = mybir.dt.int64


def _new_nc():
    # bacc.Bacc so compile() runs generate_event_semaphores -- this walrus
    # build allows at most 1 sync wait per instruction.
    return bacc.Bacc(
        "TRN2", target_bir_lowering=False, debug=False, num_devices=N_CORES
    )


# ---------------------------------------------------------------- kernel A

def build_precompute():
    """Per-core: u[t] = zT[t].T @ W1[t*128:(t+1)*128] (+ b1 if t == 0).

    Inputs : z1T [128, NP] f32 (z1 shard, transposed, padded), z2T likewise,
             W1 [256, 8] f32, b1 [1, 8] f32
    Output : u [2, NP, 8] f32 -- row r = p*KN + k holds node m = k*128 + p
    """
    nc = _new_nc()
    z1T = nc.declare_dram_parameter("z1T", [P, NP], f32, isOutput=False)
    z2T = nc.declare_dram_parameter("z2T", [P, NP], f32, isOutput=False)
    W1 = nc.declare_dram_parameter("W1", [2 * D, H], f32, isOutput=False)
    b1 = nc.declare_dram_parameter("b1", [1, H], f32, isOutput=False)
    u = nc.declare_dram_parameter("u", [2, NP, H], f32, isOutput=True)

    CH = 14                  # k-chunks per load
    CW = CH * P              # 1792 columns per load
    NLOAD = KN // CH         # 7 loads per table

    with tile.TileContext(nc) as tc:
        with (
            tc.tile_pool(name="const", bufs=1) as const_pool,
            tc.tile_pool(name="zin", bufs=3) as zin_pool,
            tc.tile_pool(name="acc", bufs=2) as acc_pool,
            tc.tile_pool(name="psum", bufs=4, space="PSUM") as psum_pool,
        ):
            w1sb = const_pool.tile([P, 2 * H], f32)   # [:, t*H:(t+1)*H] = half t
            for t in range(2):
                nc.sync.dma_start(
                    out=w1sb[:, t * H:(t + 1) * H], in_=W1[t * P:(t + 1) * P, :]
                )
            b1sb = const_pool.tile([P, H], f32)
            nc.sync.dma_start(out=b1sb[:], in_=b1[:].to_broadcast([P, H]))

            for t, zT in enumerate((z1T, z2T)):
                u_acc = acc_pool.tile([P, KN * H], f32, tag="u_acc")
                for j in range(NLOAD):
                    ztile = zin_pool.tile([P, CW], f32, tag="ztile")
                    nc.sync.dma_start(out=ztile[:], in_=zT[:, j * CW:(j + 1) * CW])
                    ps = psum_pool.tile([P, CH * H], f32, tag="ps")
                    for i in range(CH):
                        nc.tensor.matmul(
                            out=ps[:, i * H:(i + 1) * H],
                            lhsT=ztile[:, i * P:(i + 1) * P],
                            rhs=w1sb[:, t * H:(t + 1) * H],
                            start=True, stop=True,
                        )
                    if t == 0:
                        # fold b1 into u1 during the PSUM->SBUF move
                        nc.vector.tensor_tensor(
                            out=u_acc[:, j * CH * H:(j + 1) * CH * H],
                            in0=ps[:].rearrange("p (c h) -> p c h", h=H),
                            in1=b1sb[:].unsqueeze(1).to_broadcast([P, CH, H]),
                            op=mybir.AluOpType.add,
                        )
                    else:
                        nc.vector.tensor_copy(
                            out=u_acc[:, j * CH * H:(j + 1) * CH * H], in_=ps[:]
                        )
                nc.sync.dma_start(
                    out=u[t].rearrange("(p k) h -> p (k h)", p=P),
                    in_=u_acc[:],
                )
    nc.compile()
    return nc


# ---------------------------------------------------------------- kernel B

def build_gather():
    """Per-core: out[p, j] = W2 . relu(U[idx1[p,j]] + U[idx2[p,j]]) + b2

    One indirect DMA per idx column (the only indirect-DMA shape this
    walrus/ucode build handles correctly is one index per partition,
    gathering a contiguous row per partition).

    Inputs : U [U_ROWS, 8] f32 (replicated), idx1/idx2 [128, 784] i32,
             W2 [1, 8] f32, b2 [1, 1] f32
    Output : out [128, 784] f32   (edge e = j*128 + p at [p, j])
    """
    nc = _new_nc()
    U = nc.declare_dram_parameter("U", [U_ROWS, H], f32, isOutput=False)
    idx1 = nc.declare_dram_parameter("idx1", [P, EW], i32, isOutput=False)
    idx2 = nc.declare_dram_parameter("idx2", [P, EW], i32, isOutput=False)
    W2 = nc.declare_dram_parameter("W2", [1, H], f32, isOutput=False)
    b2 = nc.declare_dram_parameter("b2", [1, 1], f32, isOutput=False)
    out = nc.declare_dram_parameter("out", [P, EW], f32, isOutput=True)

    with tile.TileContext(nc) as tc:
        with (
            tc.tile_pool(name="const", bufs=1) as const_pool,
            tc.tile_pool(name="big", bufs=1) as big_pool,
        ):
            idx1s = const_pool.tile([P, EW], i32)
            nc.sync.dma_start(out=idx1s[:], in_=idx1[:])
            idx2s = const_pool.tile([P, EW], i32)
            nc.sync.dma_start(out=idx2s[:], in_=idx2[:])
            w2sb = const_pool.tile([P, H], f32)
            nc.sync.dma_start(out=w2sb[:], in_=W2[:].to_broadcast([P, H]))
            b2sb = const_pool.tile([P, 1], f32)
            nc.sync.dma_start(out=b2sb[:], in_=b2[:].to_broadcast([P, 1]))

            X1 = big_pool.tile([P, EW * H], f32, tag="X1")
            X2 = big_pool.tile([P, EW * H], f32, tag="X2")
            for j in range(EW):
                nc.gpsimd.indirect_dma_start(
                    out=X1[:, j * H:(j + 1) * H],
                    out_offset=None,
                    in_=U[:],
                    in_offset=bass.IndirectOffsetOnAxis(
                        ap=idx1s[:, j:j + 1], axis=0
                    ),
                )
                nc.gpsimd.indirect_dma_start(
                    out=X2[:, j * H:(j + 1) * H],
                    out_offset=None,
                    in_=U[:],
                    in_offset=bass.IndirectOffsetOnAxis(
                        ap=idx2s[:, j:j + 1], axis=0
                    ),
                )
            out_acc = const_pool.tile([P, EW], f32)
            CW = EW // 8          # compute chunk: 98 columns
            for c in range(8):
                s0, s1 = c * CW * H, (c + 1) * CW * H
                nc.vector.tensor_tensor(
                    out=X1[:, s0:s1], in0=X1[:, s0:s1], in1=X2[:, s0:s1],
                    op=mybir.AluOpType.add,
                )
                nc.scalar.activation(
                    out=X1[:, s0:s1], in_=X1[:, s0:s1],
                    func=mybir.ActivationFunctionType.Relu,
                )
                nc.vector.tensor_tensor(
                    out=X1[:, s0:s1].rearrange("p (w h) -> p w h", h=H),
                    in0=X1[:, s0:s1].rearrange("p (w h) -> p w h", h=H),
                    in1=w2sb[:].unsqueeze(1).to_broadcast([P, CW, H]),
                    op=mybir.AluOpType.mult,
                )
                nc.vector.tensor_reduce(
                    out=out_acc[:, c * CW:(c + 1) * CW],
                    in_=X1[:, s0:s1].rearrange("p (w h) -> p w h", h=H),
                    axis=mybir.AxisListType.X,
                    op=mybir.AluOpType.add,
                )
            nc.vector.tensor_tensor(
                out=out_acc[:],
                in0=out_acc[:],
                in1=b2sb[:].to_broadcast([P, EW]),
                op=mybir.AluOpType.add,
            )
            nc.sync.dma_start(out=out[:], in_=out_acc[:])
    nc.compile()
    return nc


# ---------------------------------------------------------------- host glue

def edge_layout(a):
    """[EC] int array -> [128, 784] per-core layout (edge e = j*128+p at [p, j])."""
    a = np.pad(a, (0, EP - EC))
    return np.ascontiguousarray(a.reshape(EW, P).T)


def inv_edge_layout(o):
    """[128, 784] kernel output -> [EC] edge-ordered values."""
    return np.ascontiguousarray(o.T.reshape(EP)[:EC])


def node_to_urow(n, table):
    """Original node ids -> U row ids for table 0 (u1) or 1 (u2)."""
    c = n // NC_NODES
    m = n % NC_NODES
    return table * (N_CORES * NP) + c * NP + (m % P) * KN + m // P


def prep_precompute_inputs(z1, z2, W1, b1):
    W1 = np.ascontiguousarray(W1, dtype=np.float32)
    b1 = np.ascontiguousarray(b1, dtype=np.float32).reshape(1, H)
    in_maps = []
    for c in range(N_CORES):
        m = {}
        for name, z in (("z1T", z1), ("z2T", z2)):
            sh = np.zeros((NP, D), dtype=np.float32)
            sh[:NC_NODES] = z[c * NC_NODES:(c + 1) * NC_NODES]
            m[name] = np.ascontiguousarray(sh.T)
        m["W1"] = W1
        m["b1"] = b1
        in_maps.append(m)
    return in_maps


def prep_gather_inputs(U, edge_index, W2, b2):
    row = node_to_urow(np.asarray(edge_index[0], dtype=np.int64), 0)
    col = node_to_urow(np.asarray(edge_index[1], dtype=np.int64), 1)
    W2 = np.ascontiguousarray(np.asarray(W2, dtype=np.float32).reshape(H)[None, :])
    b2 = np.ascontiguousarray(np.asarray(b2, dtype=np.float32)).reshape(1, 1)
    in_maps = []
    for c in range(N_CORES):
        sl = slice(c * EC, (c + 1) * EC)
        in_maps.append({
            "U": U,
            "idx1": edge_layout(row[sl]).astype(np.int32),
            "idx2": edge_layout(col[sl]).astype(np.int32),
            "W2": W2,
            "b2": b2,
        })
    return in_maps


def assemble_u(results):
    parts = [results[c]["u"][0] for c in range(N_CORES)]
    parts += [results[c]["u"][1] for c in range(N_CORES)]
    return np.ascontiguousarray(np.concatenate(parts, axis=0))


def assemble_out(results):
    outs = [inv_edge_layout(results[c]["out"]) for c in range(N_CORES)]
    return np.concatenate(outs, axis=0)[:, None].astype(np.float32)


# ---------------------------------------------------------------- entry

_CACHE = {}


def _get_kernels():
    if "a" not in _CACHE:
        _CACHE["a"] = build_precompute()
        _CACHE["b"] = build_gather()
    return _CACHE["a"], _CACHE["b"]


def run_two_phase(z1, z2, edge_index, W1, b1, W2, b2, trace=False):
    """Returns (output [N_EDGES, 1] f32, results_a, results_b)."""
    nc_a, nc_b = _get_kernels()
    core_ids = list(range(N_CORES))
    in_maps_a = prep_precompute_inputs(z1, z2, W1, b1)
    res_a = run_bass_kernel_spmd(nc_a, in_maps_a, core_ids, trace=trace)
    U = assemble_u(res_a.results)
    in_maps_b = prep_gather_inputs(U, edge_index, W2, b2)
    res_b = run_bass_kernel_spmd(nc_b, in_maps_b, core_ids, trace=trace)
    return assemble_out(res_b.results), res_a, res_b


def kernel(z1, z2, edge_index, W1, b1, W2, b2):
    z1 = np.asarray(z1, dtype=np.float32)
    z2 = np.asarray(z2, dtype=np.float32)
    edge_index = np.asarray(edge_index)
    out, _, _ = run_two_phase(z1, z2, edge_index, W1, b1, W2, b2)
    return out
